# revision 3
# baseline (speedup 1.0000x reference)
"""COPACRR forward pass on 8 Trainium2 NeuronCores (Bass kernel).

Strategy
--------
Pure data parallel over the batch dim (16 batches per core). The frozen
embedding table is uploaded once (fp16, split + padded for int16 SWDGE
gather indices) and kept device-resident; per call only the word indices
and a few small scalar tables cross the tunnel (<1MB), plus a [NG,128,32]
feature tensor coming back per core. The per-core Bass kernel does:

  - gpsimd.dma_gather(transpose=True) of doc/query embeddings straight
    into [E-partition, doc-free] fp16 layout (two gathers per tensor: the
    50k-row table is split in half so indices fit int16; out-of-half
    indices point at zero rows and the two gathers are summed)
  - PE matmuls for the q x d dot products (fp32 PSUM)
  - doc/context norms via ACT Square + ones-vector PE reduction;
    sliding-window context sums via DVE tensor_tensor_scan + shifted sub
  - cosine normalisation with row-broadcast via a constant selection
    matmul; per-query scale folded in with scalar_tensor_tensor
  - the three PACRR convs as per-filter fused multiply-accumulate over
    shifted views (q-shifts via DMA-shifted copies), relu+filter-max by
    max-accumulation (the 1x1 conv collapses to its upper envelope,
    computed host-side)
  - all top-k via the DVE top-8 instruction (InstMax, keeps duplicates,
    matching jax.lax.top_k tie semantics)

The 13-feature MLP head (208->32->32->1) runs on the host in fp32.
"""
import numpy as np

Q = 16
D = 800
E = 300
EP = 384          # padded embedding cols (fp16 row = 768B, %256==0)
DP = 896          # padded doc count per batch (7*128)
NSPLIT = 25000    # table split for int16 gather indices
ZLO = NSPLIT      # zero row (local) in lo half
ZHI = 25001       # zero row (local) in hi half
NL1 = 12          # envelope line slots for the 1x1 conv
NSCAL = 2 * NL1 + 5 * 32 + 10 * 32
OFF2 = 2 * NL1
OFF3 = OFF2 + 5 * 32
import os as _os
NCORES = int(_os.environ.get("COPA_NCORES", "8"))
B_TOTAL = 128
NB = B_TOTAL // NCORES
NG = NB // 8

_rt = {}


# ---------------------------------------------------------------------------
# host-side prep
# ---------------------------------------------------------------------------

def _make_table(emb_table):
    T = np.zeros((50003, EP), np.float16)
    n = emb_table.shape[0]          # 50001
    T[0:NSPLIT, 0:E] = emb_table[0:NSPLIT]
    T[NSPLIT + 1:NSPLIT + 1 + (n - NSPLIT), 0:E] = emb_table[NSPLIT:n]
    return T


def _split_idx(w):
    lo = np.where(w < NSPLIT, w, ZLO).astype(np.int16)
    hi = np.where(w >= NSPLIT, w - NSPLIT, ZHI).astype(np.int16)
    return lo, hi


def _wrap(idx):
    b, n = idx.shape
    return np.ascontiguousarray(idx.reshape(b, n // 16, 16).transpose(2, 0, 1))


def _doc_indices(doc_words):
    nb = doc_words.shape[0]
    lo, hi = _split_idx(doc_words)
    lo_p = np.full((nb, DP), ZLO, np.int16)
    hi_p = np.full((nb, DP), ZHI, np.int16)
    lo_p[:, :D] = lo
    hi_p[:, :D] = hi
    return _wrap(lo_p), _wrap(hi_p)


def _q_indices(qrls_words):
    nb = qrls_words.shape[0]
    lo, hi = _split_idx(qrls_words.reshape(1, nb * Q))
    return _wrap(lo)[:, 0, :], _wrap(hi)[:, 0, :]


def _upper_envelope(ws, bs):
    best = {}
    for w, b in [(0.0, 0.0)] + list(zip(ws.tolist(), bs.tolist())):
        if w not in best or b > best[w]:
            best[w] = b
    pts = sorted(best.items())
    hull = []
    for p in pts:
        while len(hull) >= 2:
            o, a = hull[-2], hull[-1]
            if (a[0] - o[0]) * (p[1] - o[1]) - (a[1] - o[1]) * (p[0] - o[0]) >= 0:
                hull.pop()
            else:
                break
        hull.append(p)
    return hull


def _make_scal(c1w, c1b, c2w, c2b, c3w, c3b):
    s = np.zeros((16, NSCAL), np.float32)
    hull = _upper_envelope(c1w.reshape(32).astype(np.float64),
                           c1b.astype(np.float64))
    if len(hull) > NL1:          # extremely unlikely; keep best-slope spread
        raise RuntimeError(f"1x1 envelope needs {len(hull)} > {NL1} lines")
    for k, (w, b) in enumerate(hull):
        s[:, 2 * k] = w
        s[:, 2 * k + 1] = b
    qpos = np.arange(16)
    m1 = (qpos + 1 <= 15).astype(np.float32)
    m2 = (qpos + 2 <= 15).astype(np.float32)
    w2 = c2w.reshape(32, 2, 2)
    for f in range(32):
        base = OFF2 + 5 * f
        s[:, base + 0] = w2[f, 0, 0]
        s[:, base + 1] = c2b[f]
        s[:, base + 2] = w2[f, 0, 1]
        s[:, base + 3] = w2[f, 1, 0] * m1
        s[:, base + 4] = w2[f, 1, 1] * m1
    w3 = c3w.reshape(32, 3, 3)
    for f in range(32):
        base = OFF3 + 10 * f
        s[:, base + 0] = w3[f, 0, 0]
        s[:, base + 1] = c3b[f]
        s[:, base + 2] = w3[f, 0, 1]
        s[:, base + 3] = w3[f, 0, 2]
        s[:, base + 4] = w3[f, 1, 0] * m1
        s[:, base + 5] = w3[f, 1, 1] * m1
        s[:, base + 6] = w3[f, 1, 2] * m1
        s[:, base + 7] = w3[f, 2, 0] * m2
        s[:, base + 8] = w3[f, 2, 1] * m2
        s[:, base + 9] = w3[f, 2, 2] * m2
    return s


def _make_rq(qn):
    """qn [nb, 16] query norms -> [128, NG] per-partition reciprocal scale."""
    nb = qn.shape[0]
    rq = 1.0 / (qn + 1e-9)
    out = np.zeros((128, nb // 8), np.float32)
    for bl in range(nb):
        out[16 * (bl % 8):16 * (bl % 8) + 16, bl // 8] = rq[bl]
    return out


# ---------------------------------------------------------------------------
# bass kernel
# ---------------------------------------------------------------------------

def _emit(nc, table, ilo, ihi, qlo, qhi, scal16, rq):
    import concourse.mybir as mybir
    import concourse.tile as tile

    AL = mybir.AluOpType
    AF = mybir.ActivationFunctionType
    F16, F32, I16 = mybir.dt.float16, mybir.dt.float32, mybir.dt.int16
    HALVES = ((0, 512), (512, 800))
    NQ = 16 * NB
    NI = DP // 16
    out = nc.dram_tensor("feats", [NG, 128, 32], F16, kind="ExternalOutput")

    with tile.TileContext(nc) as tc:
        with (
            tc.tile_pool(name="const", bufs=1) as cp,
            tc.tile_pool(name="batch", bufs=2) as bp,
            tc.tile_pool(name="grp", bufs=2) as gp,
            tc.tile_pool(name="conv", bufs=2) as vp,
            tc.tile_pool(name="praw", bufs=1, space="PSUM") as ppraw,
            tc.tile_pool(name="pdn", bufs=1, space="PSUM") as ppdn,
            tc.tile_pool(name="pcn", bufs=1, space="PSUM") as ppcn,
            tc.tile_pool(name="pbc", bufs=1, space="PSUM") as ppbc,
        ):
            t_ilo = cp.tile([128, NB, NI], I16)
            t_ihi = cp.tile([128, NB, NI], I16)
            t_qlo = cp.tile([128, NB], I16)
            t_qhi = cp.tile([128, NB], I16)
            # HW's SWDGE idx stream reads partitions 16..31; CoreSim reads 0..15.
            for t, src in ((t_ilo, ilo), (t_ihi, ihi), (t_qlo, qlo), (t_qhi, qhi)):
                nc.vector.memset(t[:], 0)
                nc.sync.dma_start(t[0:16], src[:])
                nc.sync.dma_start(t[16:32], src[:])
            t_scal16 = cp.tile([16, NSCAL], F32)
            nc.sync.dma_start(t_scal16[:], scal16[:])
            t_rq = cp.tile([128, NG], F32)
            nc.sync.dma_start(t_rq[:], rq[:])

            t_scal = cp.tile([128, NSCAL], F32)
            for k in range(8):
                nc.sync.dma_start(t_scal[16 * k:16 * k + 16, :], scal16[:])

            # selR8[k, m] = (m // 16 == k): broadcasts row k to partitions 16k..
            t_sel8 = cp.tile([8, 128], F32)
            nc.gpsimd.memset(t_sel8[:], 1.0)
            nc.gpsimd.affine_select(out=t_sel8[:], in_=t_sel8[:],
                                    compare_op=AL.is_ge, fill=0.0, base=0,
                                    pattern=[[1, 128]], channel_multiplier=-16)
            nc.gpsimd.affine_select(out=t_sel8[:], in_=t_sel8[:],
                                    compare_op=AL.is_ge, fill=0.0, base=15,
                                    pattern=[[-1, 128]], channel_multiplier=16)

            t_ones128 = cp.tile([128, 1], F16)
            nc.vector.memset(t_ones128[:], 1.0)

            t_qloT = cp.tile([128, 3, NQ], F16, tag="qT")
            t_qhiT = cp.tile([128, 3, NQ], F16, tag="qTh")
            nc.gpsimd.dma_gather(t_qloT[:], table[0:NSPLIT + 1, :], t_qlo[:],
                                 NQ, NQ, EP, transpose=True)
            nc.gpsimd.dma_gather(t_qhiT[:], table[NSPLIT + 1:50003, :], t_qhi[:],
                                 NQ, NQ, EP, transpose=True)
            nc.vector.tensor_tensor(out=t_qloT[:], in0=t_qloT[:], in1=t_qhiT[:],
                                    op=AL.add)
            t_qT = t_qloT

            for g in range(NG):
                raws = gp.tile([128, 800], F32, tag="raws")
                sdn = gp.tile([8, 800], F32, tag="sdn")
                scn = gp.tile([8, 800], F32, tag="scn")

                for r in range(8):
                    bl = 8 * g + r
                    dlo = bp.tile([128, 3, DP], F16, tag="dlo")
                    dhi = bp.tile([128, 3, DP], F16, tag="dhi")
                    nc.gpsimd.dma_gather(dlo[:], table[0:NSPLIT + 1, :],
                                         t_ilo[:, bl, :], DP, DP, EP,
                                         transpose=True)
                    nc.gpsimd.dma_gather(dhi[:], table[NSPLIT + 1:50003, :],
                                         t_ihi[:, bl, :], DP, DP, EP,
                                         transpose=True)
                    nc.vector.tensor_tensor(out=dlo[:], in0=dlo[:], in1=dhi[:],
                                            op=AL.add)

                    ps_raw = ppraw.tile([16, 800], F32, tag="praw")
                    for c in range(3):
                        for (s0, s1) in HALVES:
                            nc.tensor.matmul(
                                ps_raw[:, s0:s1],
                                t_qT[:, c, 16 * bl:16 * bl + 16],
                                dlo[:, c, s0:s1],
                                start=(c == 0), stop=(c == 2))
                    raw_st = bp.tile([16, 800], F32, tag="raw_st")
                    nc.vector.tensor_copy(raw_st[:], ps_raw[:])
                    nc.sync.dma_start(raws[16 * r:16 * r + 16, :], raw_st[:])

                    sq = bp.tile([128, 3, D], F16, tag="sq")
                    nc.scalar.activation(sq[:], dlo[:, :, 0:D], AF.Square)
                    ps_dn = ppdn.tile([1, 800], F32, tag="pdn")
                    for c in range(3):
                        for (s0, s1) in HALVES:
                            nc.tensor.matmul(
                                ps_dn[:, s0:s1], t_ones128[:],
                                sq[:, c, s0:s1], start=(c == 0), stop=(c == 2))
                    dn_st = bp.tile([1, 800], F32, tag="dn_st")
                    nc.vector.tensor_copy(dn_st[:], ps_dn[:])
                    nc.sync.dma_start(sdn[r:r + 1, :], dn_st[:])

                    csum = bp.tile([128, 3, 809], F32, tag="csum")
                    nc.vector.memset(csum[:, :, 0:5], 0.0)
                    for c in range(3):
                        nc.vector.tensor_tensor_scan(
                            out=csum[:, c, 5:805], data0=dlo[:, c, 0:D],
                            data1=dlo[:, c, 0:D], initial=0.0,
                            op0=AL.add, op1=AL.bypass)
                    ctxs = bp.tile([128, 3, D], F32, tag="ctxs")
                    nc.vector.tensor_tensor(out=ctxs[:, :, 0:797],
                                            in0=csum[:, :, 8:805],
                                            in1=csum[:, :, 0:797],
                                            op=AL.subtract)
                    for c in range(3):
                        nc.vector.tensor_scalar(
                            out=ctxs[:, c, 797:800], in0=csum[:, c, 797:800],
                            scalar1=-1.0, scalar2=csum[:, c, 804:805],
                            op0=AL.mult, op1=AL.add)
                    sqc = bp.tile([128, 3, D], F16, tag="sqc")
                    nc.scalar.activation(sqc[:], ctxs[:], AF.Square)
                    ps_cn = ppcn.tile([1, 800], F32, tag="pcn")
                    for c in range(3):
                        for (s0, s1) in HALVES:
                            nc.tensor.matmul(
                                ps_cn[:, s0:s1], t_ones128[:],
                                sqc[:, c, s0:s1], start=(c == 0), stop=(c == 2))
                    cn_st = bp.tile([1, 800], F32, tag="cn_st")
                    nc.vector.tensor_copy(cn_st[:], ps_cn[:])
                    nc.sync.dma_start(scn[r:r + 1, :], cn_st[:])

                dnr = gp.tile([8, 800], F32, tag="dnr")
                rrd = gp.tile([8, 800], F32, tag="rrd")
                rrc = gp.tile([8, 800], F32, tag="rrc")
                nc.scalar.activation(dnr[:], sdn[:], AF.Sqrt)
                nc.vector.tensor_scalar(out=dnr[:], in0=dnr[:], scalar1=1e-9,
                                        scalar2=None, op0=AL.add)
                nc.vector.reciprocal(rrd[:], dnr[:])
                nc.scalar.activation(dnr[:], scn[:], AF.Sqrt)
                nc.vector.tensor_scalar(out=dnr[:], in0=dnr[:], scalar1=9e-9,
                                        scalar2=None, op0=AL.add)
                nc.vector.reciprocal(rrc[:], dnr[:])

                ps_bc = ppbc.tile([128, 800], F32, tag="pbc")
                for (s0, s1) in HALVES:
                    nc.tensor.matmul(ps_bc[:, s0:s1], t_sel8[:], rrd[:, s0:s1],
                                     start=True, stop=True)
                cosg = gp.tile([128, 804], F16, tag="cosg")
                nc.vector.memset(cosg[:, 800:804], 0.0)
                nc.vector.scalar_tensor_tensor(
                    out=cosg[:, 0:800], in0=raws[:], scalar=t_rq[:, g:g + 1],
                    in1=ps_bc[:], op0=AL.mult, op1=AL.mult)

                csn = gp.tile([128, 809], F32, tag="csn")
                nc.vector.memset(csn[:, 0:5], 0.0)
                nc.vector.tensor_tensor_scan(
                    out=csn[:, 5:805], data0=raws[:], data1=raws[:],
                    initial=0.0, op0=AL.add, op1=AL.bypass)
                ctxn = gp.tile([128, 800], F32, tag="ctxn")
                nc.vector.tensor_tensor(out=ctxn[:, 0:797], in0=csn[:, 8:805],
                                        in1=csn[:, 0:797], op=AL.subtract)
                nc.vector.tensor_scalar(out=ctxn[:, 797:800],
                                        in0=csn[:, 797:800],
                                        scalar1=-1.0, scalar2=csn[:, 804:805],
                                        op0=AL.mult, op1=AL.add)
                ps_bc2 = ppbc.tile([128, 800], F32, tag="pbc")
                for (s0, s1) in HALVES:
                    nc.tensor.matmul(ps_bc2[:, s0:s1], t_sel8[:], rrc[:, s0:s1],
                                     start=True, stop=True)
                coc = gp.tile([128, 800], F16, tag="coc")
                nc.vector.scalar_tensor_tensor(
                    out=coc[:], in0=ctxn[:], scalar=t_rq[:, g:g + 1],
                    in1=ps_bc2[:], op0=AL.mult, op1=AL.mult)

                cosgs1 = gp.tile([128, 804], F16, tag="cosgs1")
                cosgs2 = gp.tile([128, 804], F16, tag="cosgs2")
                nc.sync.dma_start(cosgs1[0:127, :], cosg[1:128, :])
                nc.sync.dma_start(cosgs2[0:126, :], cosg[2:128, :])

                fe = gp.tile([128, 32], F16, tag="fe")
                nc.vector.max(fe[:, 24:32], coc[:])

                topf1 = vp.tile([128, 800], F16, tag="topf1")
                tmp1 = vp.tile([128, 800], F16, tag="tmp1")
                nc.vector.tensor_scalar(out=topf1[:], in0=cosg[:, 0:800],
                                        scalar1=t_scal[:, 0:1],
                                        scalar2=t_scal[:, 1:2],
                                        op0=AL.mult, op1=AL.add)
                for k in range(1, NL1):
                    nc.vector.tensor_scalar(out=tmp1[:], in0=cosg[:, 0:800],
                                            scalar1=t_scal[:, 2 * k:2 * k + 1],
                                            scalar2=t_scal[:, 2 * k + 1:2 * k + 2],
                                            op0=AL.mult, op1=AL.add)
                    nc.vector.tensor_tensor(out=topf1[:], in0=topf1[:],
                                            in1=tmp1[:], op=AL.max)
                nc.vector.max(fe[:, 0:8], topf1[:])

                topf2 = vp.tile([128, 800], F16, tag="topf2")
                acc2 = vp.tile([128, 800], F16, tag="acc2")
                for f in range(32):
                    base = OFF2 + 5 * f
                    nc.vector.tensor_scalar(out=acc2[:], in0=cosg[:, 0:800],
                                            scalar1=t_scal[:, base:base + 1],
                                            scalar2=t_scal[:, base + 1:base + 2],
                                            op0=AL.mult, op1=AL.add)
                    nc.vector.scalar_tensor_tensor(
                        out=acc2[:], in0=cosg[:, 1:801],
                        scalar=t_scal[:, base + 2:base + 3], in1=acc2[:],
                        op0=AL.mult, op1=AL.add)
                    nc.vector.scalar_tensor_tensor(
                        out=acc2[0:127, :], in0=cosgs1[0:127, 0:800],
                        scalar=t_scal[0:127, base + 3:base + 4],
                        in1=acc2[0:127, :], op0=AL.mult, op1=AL.add)
                    nc.vector.scalar_tensor_tensor(
                        out=acc2[0:127, :], in0=cosgs1[0:127, 1:801],
                        scalar=t_scal[0:127, base + 4:base + 5],
                        in1=acc2[0:127, :], op0=AL.mult, op1=AL.add)
                    if f == 0:
                        nc.vector.tensor_scalar(out=topf2[:], in0=acc2[:],
                                                scalar1=0.0, scalar2=None,
                                                op0=AL.max)
                    else:
                        nc.vector.tensor_tensor(out=topf2[:], in0=topf2[:],
                                                in1=acc2[:], op=AL.max)
                nc.vector.max(fe[:, 8:16], topf2[:])

                topf3 = vp.tile([128, 800], F16, tag="topf3")
                acc3 = vp.tile([128, 800], F16, tag="acc3")
                for f in range(32):
                    base = OFF3 + 10 * f
                    nc.vector.tensor_scalar(out=acc3[:], in0=cosg[:, 0:800],
                                            scalar1=t_scal[:, base:base + 1],
                                            scalar2=t_scal[:, base + 1:base + 2],
                                            op0=AL.mult, op1=AL.add)
                    for c in (1, 2):
                        nc.vector.scalar_tensor_tensor(
                            out=acc3[:], in0=cosg[:, c:800 + c],
                            scalar=t_scal[:, base + 1 + c:base + 2 + c],
                            in1=acc3[:], op0=AL.mult, op1=AL.add)
                    for a, csrc in ((1, cosgs1), (2, cosgs2)):
                        pmax = 128 - a
                        for c in (0, 1, 2):
                            col = base + 1 + 3 * a + c
                            nc.vector.scalar_tensor_tensor(
                                out=acc3[0:pmax, :], in0=csrc[0:pmax, c:800 + c],
                                scalar=t_scal[0:pmax, col:col + 1],
                                in1=acc3[0:pmax, :], op0=AL.mult, op1=AL.add)
                    if f == 0:
                        nc.vector.tensor_scalar(out=topf3[:], in0=acc3[:],
                                                scalar1=0.0, scalar2=None,
                                                op0=AL.max)
                    else:
                        nc.vector.tensor_tensor(out=topf3[:], in0=topf3[:],
                                                in1=acc3[:], op=AL.max)
                nc.vector.max(fe[:, 16:24], topf3[:])

                nc.sync.dma_start(out[g, :, :], fe[:])

    return out


# ---------------------------------------------------------------------------
# runtime
# ---------------------------------------------------------------------------

def _table_fp(emb):
    return (emb.shape, float(emb[0, 0]), float(emb[-1, -1]),
            float(emb[::971, ::7].sum()))


def _get_runtime(emb):
    fp = _table_fp(emb)
    if _rt.get("fp") == fp:
        return _rt
    import jax
    from jax.sharding import Mesh, PartitionSpec as P, NamedSharding
    from concourse.bass2jax import bass_jit, bass_shard_map

    devs = jax.devices()[:NCORES]
    mesh = Mesh(np.asarray(devs), ("core",))
    rep = NamedSharding(mesh, P())
    table = _make_table(emb)
    table_dev = jax.jit(lambda x: x, in_shardings=rep, out_shardings=rep)(table)
    table_dev.block_until_ready()

    if "fn" not in _rt:
        _rt["fn"] = bass_shard_map(
            bass_jit(_emit), mesh=mesh,
            in_specs=(P(), P("core"), P("core"), P("core"), P("core"),
                      P("core"), P("core")),
            out_specs=P("core"))
    _rt["table_dev"] = table_dev
    _rt["fp"] = fp
    return _rt


def kernel(qrls_words, doc_words, emb_table, idf_table,
           conv1_w, conv1_b, conv2_w, conv2_b, conv3_w, conv3_b,
           w1, b1, w2, b2, w3, b3):
    qw = np.asarray(qrls_words).astype(np.int64)
    dw = np.asarray(doc_words).astype(np.int64)
    emb = np.asarray(emb_table, np.float32)
    idf_t = np.asarray(idf_table, np.float32)
    B = qw.shape[0]
    assert B == B_TOTAL and qw.shape[1] == Q and dw.shape[1] == D

    rt = _get_runtime(emb)

    ilo_l, ihi_l, qlo_l, qhi_l, rq_l = [], [], [], [], []
    for k in range(NCORES):
        sl = slice(NB * k, NB * (k + 1))
        a, b_ = _doc_indices(dw[sl])
        c, d_ = _q_indices(qw[sl])
        ilo_l.append(a); ihi_l.append(b_); qlo_l.append(c); qhi_l.append(d_)
        qn = np.linalg.norm(emb[qw[sl]], axis=2)
        rq_l.append(_make_rq(qn))
    scal = _make_scal(np.asarray(conv1_w, np.float32), np.asarray(conv1_b, np.float32),
                      np.asarray(conv2_w, np.float32), np.asarray(conv2_b, np.float32),
                      np.asarray(conv3_w, np.float32), np.asarray(conv3_b, np.float32))

    feats = rt["fn"](rt["table_dev"],
                     np.concatenate(ilo_l, 0), np.concatenate(ihi_l, 0),
                     np.concatenate(qlo_l, 0), np.concatenate(qhi_l, 0),
                     np.tile(scal, (NCORES, 1)), np.concatenate(rq_l, 0))
    fnp = np.asarray(feats).astype(np.float32)     # [NCORES*NG, 128, 32]

    idf = idf_t[qw]                                # [B, 16]
    scores = np.zeros((B, Q, 13), np.float32)
    f5 = fnp.reshape(NCORES * NG, 8, 16, 32)
    for k in range(NCORES):
        for g in range(NG):
            for r in range(8):
                bl = NB * k + 8 * g + r
                blk = f5[NG * k + g, r]
                scores[bl, :, 0:2] = blk[:, 0:2]
                scores[bl, :, 2:4] = blk[:, 8:10]
                scores[bl, :, 4:6] = blk[:, 16:18]
                scores[bl, :, 6:12] = blk[:, 24:30]
                scores[bl, :, 12] = idf[bl]

    x = scores.reshape(B, -1)
    x = np.maximum(x @ np.asarray(w1, np.float32) + np.asarray(b1, np.float32), 0)
    x = np.maximum(x @ np.asarray(w2, np.float32) + np.asarray(b2, np.float32), 0)
    return x @ np.asarray(w3, np.float32) + np.asarray(b3, np.float32)


# revision 4
# speedup vs baseline: 1.7329x; 1.7329x over previous
"""COPACRR forward pass on 8 Trainium2 NeuronCores (Bass kernel).

Strategy
--------
Pure data parallel over the batch dim (16 batches per core). The frozen
embedding table is uploaded once (fp16, split + padded for int16 SWDGE
gather indices) and kept device-resident; per call only the word indices
and a few small scalar tables cross the tunnel (<1MB), plus a [NG,128,32]
feature tensor coming back per core. The per-core Bass kernel does:

  - gpsimd.dma_gather(transpose=True) of doc/query embeddings straight
    into [E-partition, doc-free] fp16 layout (two gathers per tensor: the
    50k-row table is split in half so indices fit int16; out-of-half
    indices point at zero rows and the two gathers are summed)
  - PE matmuls for the q x d dot products (fp32 PSUM)
  - doc/context norms via ACT Square + ones-vector PE reduction;
    sliding-window context sums via DVE tensor_tensor_scan + shifted sub
  - cosine normalisation with row-broadcast via a constant selection
    matmul; per-query scale folded in with scalar_tensor_tensor
  - the three PACRR convs as per-filter fused multiply-accumulate over
    shifted views (q-shifts via DMA-shifted copies), relu+filter-max by
    max-accumulation (the 1x1 conv collapses to its upper envelope,
    computed host-side)
  - all top-k via the DVE top-8 instruction (InstMax, keeps duplicates,
    matching jax.lax.top_k tie semantics)

The 13-feature MLP head (208->32->32->1) runs on the host in fp32.
"""
import numpy as np

Q = 16
D = 800
E = 300
EP = 384          # padded embedding cols (fp16 row = 768B, %256==0)
DP = 896          # padded doc count per batch (7*128)
NSPLIT = 25000    # table split for int16 gather indices
ZLO = NSPLIT      # zero row (local) in lo half
ZHI = 25001       # zero row (local) in hi half
NL1 = 12          # envelope line slots for the 1x1 conv
NSCAL = 2 * NL1 + 5 * 32 + 10 * 32
OFF2 = 2 * NL1
OFF3 = OFF2 + 5 * 32
import os as _os
NCORES = int(_os.environ.get("COPA_NCORES", "8"))
B_TOTAL = 128
NB = B_TOTAL // NCORES
NG = NB // 8

_rt = {}


# ---------------------------------------------------------------------------
# host-side prep
# ---------------------------------------------------------------------------

def _make_table(emb_table):
    T = np.zeros((50003, EP), np.float16)
    n = emb_table.shape[0]          # 50001
    T[0:NSPLIT, 0:E] = emb_table[0:NSPLIT]
    T[NSPLIT + 1:NSPLIT + 1 + (n - NSPLIT), 0:E] = emb_table[NSPLIT:n]
    return T


def _split_idx(w):
    lo = np.where(w < NSPLIT, w, ZLO).astype(np.int16)
    hi = np.where(w >= NSPLIT, w - NSPLIT, ZHI).astype(np.int16)
    return lo, hi


def _wrap(idx):
    b, n = idx.shape
    return np.ascontiguousarray(idx.reshape(b, n // 16, 16).transpose(2, 0, 1))


def _doc_indices(doc_words):
    nb = doc_words.shape[0]
    lo, hi = _split_idx(doc_words)
    lo_p = np.full((nb, DP), ZLO, np.int16)
    hi_p = np.full((nb, DP), ZHI, np.int16)
    lo_p[:, :D] = lo
    hi_p[:, :D] = hi
    return _wrap(lo_p), _wrap(hi_p)


def _q_indices(qrls_words):
    nb = qrls_words.shape[0]
    lo, hi = _split_idx(qrls_words.reshape(1, nb * Q))
    return _wrap(lo)[:, 0, :], _wrap(hi)[:, 0, :]


def _upper_envelope(ws, bs):
    best = {}
    for w, b in [(0.0, 0.0)] + list(zip(ws.tolist(), bs.tolist())):
        if w not in best or b > best[w]:
            best[w] = b
    pts = sorted(best.items())
    hull = []
    for p in pts:
        while len(hull) >= 2:
            o, a = hull[-2], hull[-1]
            if (a[0] - o[0]) * (p[1] - o[1]) - (a[1] - o[1]) * (p[0] - o[0]) >= 0:
                hull.pop()
            else:
                break
        hull.append(p)
    return hull


def _make_scal(c1w, c1b, c2w, c2b, c3w, c3b):
    s = np.zeros((16, NSCAL), np.float32)
    hull = _upper_envelope(c1w.reshape(32).astype(np.float64),
                           c1b.astype(np.float64))
    if len(hull) > NL1:          # extremely unlikely; keep best-slope spread
        raise RuntimeError(f"1x1 envelope needs {len(hull)} > {NL1} lines")
    for k, (w, b) in enumerate(hull):
        s[:, 2 * k] = w
        s[:, 2 * k + 1] = b
    qpos = np.arange(16)
    m1 = (qpos + 1 <= 15).astype(np.float32)
    m2 = (qpos + 2 <= 15).astype(np.float32)
    w2 = c2w.reshape(32, 2, 2)
    for f in range(32):
        base = OFF2 + 5 * f
        s[:, base + 0] = w2[f, 0, 0]
        s[:, base + 1] = c2b[f]
        s[:, base + 2] = w2[f, 0, 1]
        s[:, base + 3] = w2[f, 1, 0] * m1
        s[:, base + 4] = w2[f, 1, 1] * m1
    w3 = c3w.reshape(32, 3, 3)
    for f in range(32):
        base = OFF3 + 10 * f
        s[:, base + 0] = w3[f, 0, 0]
        s[:, base + 1] = c3b[f]
        s[:, base + 2] = w3[f, 0, 1]
        s[:, base + 3] = w3[f, 0, 2]
        s[:, base + 4] = w3[f, 1, 0] * m1
        s[:, base + 5] = w3[f, 1, 1] * m1
        s[:, base + 6] = w3[f, 1, 2] * m1
        s[:, base + 7] = w3[f, 2, 0] * m2
        s[:, base + 8] = w3[f, 2, 1] * m2
        s[:, base + 9] = w3[f, 2, 2] * m2
    return s


def _make_rq(qn):
    """qn [nb, 16] query norms -> [128, NG] per-partition reciprocal scale."""
    nb = qn.shape[0]
    rq = 1.0 / (qn + 1e-9)
    out = np.zeros((128, nb // 8), np.float32)
    for bl in range(nb):
        out[16 * (bl % 8):16 * (bl % 8) + 16, bl // 8] = rq[bl]
    return out


# ---------------------------------------------------------------------------
# bass kernel
# ---------------------------------------------------------------------------

def _emit(nc, table, ilo, ihi, qlo, qhi, scal16, rq):
    import concourse.mybir as mybir
    import concourse.tile as tile

    AL = mybir.AluOpType
    AF = mybir.ActivationFunctionType
    F16, F32, I16 = mybir.dt.float16, mybir.dt.float32, mybir.dt.int16
    HALVES = ((0, 512), (512, 800))
    NQ = 16 * NB
    NI = DP // 16
    out = nc.dram_tensor("feats", [NG, 128, 32], F16, kind="ExternalOutput")

    with tile.TileContext(nc) as tc:
        with (
            tc.tile_pool(name="const", bufs=1) as cp,
            tc.tile_pool(name="batch", bufs=2) as bp,
            tc.tile_pool(name="grp", bufs=2) as gp,
            tc.tile_pool(name="conv", bufs=2) as vp,
            tc.tile_pool(name="praw", bufs=1, space="PSUM") as ppraw,
            tc.tile_pool(name="pdn", bufs=1, space="PSUM") as ppdn,
            tc.tile_pool(name="pcn", bufs=1, space="PSUM") as ppcn,
            tc.tile_pool(name="pbc", bufs=1, space="PSUM") as ppbc,
        ):
            t_ilo = cp.tile([128, NB, NI], I16)
            t_ihi = cp.tile([128, NB, NI], I16)
            t_qlo = cp.tile([128, NB], I16)
            t_qhi = cp.tile([128, NB], I16)
            # HW's SWDGE idx stream reads partitions 16..31; CoreSim reads 0..15.
            for t, src in ((t_ilo, ilo), (t_ihi, ihi), (t_qlo, qlo), (t_qhi, qhi)):
                nc.vector.memset(t[:], 0)
                nc.sync.dma_start(t[0:16], src[:])
                nc.sync.dma_start(t[16:32], src[:])
            t_scal16 = cp.tile([16, NSCAL], F32)
            nc.sync.dma_start(t_scal16[:], scal16[:])
            t_rq = cp.tile([128, NG], F32)
            nc.sync.dma_start(t_rq[:], rq[:])

            t_scal = cp.tile([128, NSCAL], F32)
            for k in range(8):
                nc.sync.dma_start(t_scal[16 * k:16 * k + 16, :], scal16[:])

            # selR8[k, m] = (m // 16 == k): broadcasts row k to partitions 16k..
            t_sel8 = cp.tile([8, 128], F32)
            nc.gpsimd.memset(t_sel8[:], 1.0)
            nc.gpsimd.affine_select(out=t_sel8[:], in_=t_sel8[:],
                                    compare_op=AL.is_ge, fill=0.0, base=0,
                                    pattern=[[1, 128]], channel_multiplier=-16)
            nc.gpsimd.affine_select(out=t_sel8[:], in_=t_sel8[:],
                                    compare_op=AL.is_ge, fill=0.0, base=15,
                                    pattern=[[-1, 128]], channel_multiplier=16)

            t_ones128 = cp.tile([128, 1], F16)
            nc.vector.memset(t_ones128[:], 1.0)

            t_qloT = cp.tile([128, 3, NQ], F16, tag="qT")
            t_qhiT = cp.tile([128, 3, NQ], F16, tag="qTh")
            nc.gpsimd.dma_gather(t_qloT[:], table[0:NSPLIT + 1, :], t_qlo[:],
                                 NQ, NQ, EP, transpose=True)
            nc.gpsimd.dma_gather(t_qhiT[:], table[NSPLIT + 1:50003, :], t_qhi[:],
                                 NQ, NQ, EP, transpose=True)
            nc.vector.tensor_tensor(out=t_qloT[:], in0=t_qloT[:], in1=t_qhiT[:],
                                    op=AL.add)
            t_qT = t_qloT

            for g in range(NG):
                raws = gp.tile([128, 800], F32, tag="raws")
                sdn = gp.tile([8, 800], F32, tag="sdn")
                scn = gp.tile([8, 800], F32, tag="scn")

                for r in range(8):
                    bl = 8 * g + r
                    dlo = bp.tile([128, 3, DP], F16, tag="dlo")
                    dhi = bp.tile([128, 3, DP], F16, tag="dhi")
                    nc.gpsimd.dma_gather(dlo[:], table[0:NSPLIT + 1, :],
                                         t_ilo[:, bl, :], DP, DP, EP,
                                         transpose=True)
                    nc.gpsimd.dma_gather(dhi[:], table[NSPLIT + 1:50003, :],
                                         t_ihi[:, bl, :], DP, DP, EP,
                                         transpose=True)
                    nc.vector.tensor_tensor(out=dlo[:], in0=dlo[:], in1=dhi[:],
                                            op=AL.add)

                    ps_raw = ppraw.tile([16, 800], F32, tag="praw")
                    for c in range(3):
                        for (s0, s1) in HALVES:
                            nc.tensor.matmul(
                                ps_raw[:, s0:s1],
                                t_qT[:, c, 16 * bl:16 * bl + 16],
                                dlo[:, c, s0:s1],
                                start=(c == 0), stop=(c == 2))
                    raw_st = bp.tile([16, 800], F32, tag="raw_st")
                    nc.vector.tensor_copy(raw_st[:], ps_raw[:])
                    nc.sync.dma_start(raws[16 * r:16 * r + 16, :], raw_st[:])

                    sq = bp.tile([128, 3, D], F16, tag="sq")
                    nc.scalar.activation(sq[:], dlo[:, :, 0:D], AF.Square)
                    ps_dn = ppdn.tile([1, 800], F32, tag="pdn")
                    for c in range(3):
                        for (s0, s1) in HALVES:
                            nc.tensor.matmul(
                                ps_dn[:, s0:s1], t_ones128[:],
                                sq[:, c, s0:s1], start=(c == 0), stop=(c == 2))
                    dn_st = bp.tile([1, 800], F32, tag="dn_st")
                    nc.vector.tensor_copy(dn_st[:], ps_dn[:])
                    nc.sync.dma_start(sdn[r:r + 1, :], dn_st[:])

                    csum = bp.tile([128, 3, 809], F32, tag="csum")
                    nc.vector.memset(csum[:, :, 0:5], 0.0)
                    for c in range(3):
                        nc.vector.tensor_tensor_scan(
                            out=csum[:, c, 5:805], data0=dlo[:, c, 0:D],
                            data1=dlo[:, c, 0:D], initial=0.0,
                            op0=AL.add, op1=AL.bypass)
                    ctxs = bp.tile([128, 3, D], F32, tag="ctxs")
                    nc.vector.tensor_tensor(out=ctxs[:, :, 0:797],
                                            in0=csum[:, :, 8:805],
                                            in1=csum[:, :, 0:797],
                                            op=AL.subtract)
                    for c in range(3):
                        nc.vector.tensor_scalar(
                            out=ctxs[:, c, 797:800], in0=csum[:, c, 797:800],
                            scalar1=-1.0, scalar2=csum[:, c, 804:805],
                            op0=AL.mult, op1=AL.add)
                    sqc = bp.tile([128, 3, D], F16, tag="sqc")
                    nc.scalar.activation(sqc[:], ctxs[:], AF.Square)
                    ps_cn = ppcn.tile([1, 800], F32, tag="pcn")
                    for c in range(3):
                        for (s0, s1) in HALVES:
                            nc.tensor.matmul(
                                ps_cn[:, s0:s1], t_ones128[:],
                                sqc[:, c, s0:s1], start=(c == 0), stop=(c == 2))
                    cn_st = bp.tile([1, 800], F32, tag="cn_st")
                    nc.vector.tensor_copy(cn_st[:], ps_cn[:])
                    nc.sync.dma_start(scn[r:r + 1, :], cn_st[:])

                dnr = gp.tile([8, 800], F32, tag="dnr")
                rrd = gp.tile([8, 800], F32, tag="rrd")
                rrc = gp.tile([8, 800], F32, tag="rrc")
                nc.scalar.activation(dnr[:], sdn[:], AF.Sqrt)
                nc.vector.tensor_scalar(out=dnr[:], in0=dnr[:], scalar1=1e-9,
                                        scalar2=None, op0=AL.add)
                nc.vector.reciprocal(rrd[:], dnr[:])
                nc.scalar.activation(dnr[:], scn[:], AF.Sqrt)
                nc.vector.tensor_scalar(out=dnr[:], in0=dnr[:], scalar1=9e-9,
                                        scalar2=None, op0=AL.add)
                nc.vector.reciprocal(rrc[:], dnr[:])

                ps_bc = ppbc.tile([128, 800], F32, tag="pbc")
                for (s0, s1) in HALVES:
                    nc.tensor.matmul(ps_bc[:, s0:s1], t_sel8[:], rrd[:, s0:s1],
                                     start=True, stop=True)
                cosg = gp.tile([128, 804], F16, tag="cosg")
                nc.vector.memset(cosg[:, 800:804], 0.0)
                nc.vector.scalar_tensor_tensor(
                    out=cosg[:, 0:800], in0=raws[:], scalar=t_rq[:, g:g + 1],
                    in1=ps_bc[:], op0=AL.mult, op1=AL.mult)

                csn = gp.tile([128, 809], F32, tag="csn")
                nc.vector.memset(csn[:, 0:5], 0.0)
                nc.vector.tensor_tensor_scan(
                    out=csn[:, 5:805], data0=raws[:], data1=raws[:],
                    initial=0.0, op0=AL.add, op1=AL.bypass)
                ctxn = gp.tile([128, 800], F32, tag="ctxn")
                nc.vector.tensor_tensor(out=ctxn[:, 0:797], in0=csn[:, 8:805],
                                        in1=csn[:, 0:797], op=AL.subtract)
                nc.vector.tensor_scalar(out=ctxn[:, 797:800],
                                        in0=csn[:, 797:800],
                                        scalar1=-1.0, scalar2=csn[:, 804:805],
                                        op0=AL.mult, op1=AL.add)
                ps_bc2 = ppbc.tile([128, 800], F32, tag="pbc")
                for (s0, s1) in HALVES:
                    nc.tensor.matmul(ps_bc2[:, s0:s1], t_sel8[:], rrc[:, s0:s1],
                                     start=True, stop=True)
                coc = gp.tile([128, 800], F16, tag="coc")
                nc.vector.scalar_tensor_tensor(
                    out=coc[:], in0=ctxn[:], scalar=t_rq[:, g:g + 1],
                    in1=ps_bc2[:], op0=AL.mult, op1=AL.mult)

                cosgs1 = gp.tile([128, 804], F16, tag="cosgs1")
                cosgs2 = gp.tile([128, 804], F16, tag="cosgs2")
                nc.sync.dma_start(cosgs1[0:127, :], cosg[1:128, :])
                nc.sync.dma_start(cosgs2[0:126, :], cosg[2:128, :])

                fe = gp.tile([128, 32], F16, tag="fe")
                nc.vector.max(fe[:, 24:32], coc[:])

                topf1 = vp.tile([128, 800], F16, tag="topf1")
                tmp1 = vp.tile([128, 800], F16, tag="tmp1")
                nc.vector.tensor_scalar(out=topf1[:], in0=cosg[:, 0:800],
                                        scalar1=t_scal[:, 0:1],
                                        scalar2=t_scal[:, 1:2],
                                        op0=AL.mult, op1=AL.add)
                for k in range(1, NL1):
                    nc.vector.tensor_scalar(out=tmp1[:], in0=cosg[:, 0:800],
                                            scalar1=t_scal[:, 2 * k:2 * k + 1],
                                            scalar2=t_scal[:, 2 * k + 1:2 * k + 2],
                                            op0=AL.mult, op1=AL.add)
                    nc.vector.tensor_tensor(out=topf1[:], in0=topf1[:],
                                            in1=tmp1[:], op=AL.max)
                nc.vector.max(fe[:, 0:8], topf1[:])

                topf2 = vp.tile([128, 800], F16, tag="topf2")
                acc2 = vp.tile([128, 800], F16, tag="acc2")
                for f in range(32):
                    base = OFF2 + 5 * f
                    nc.vector.tensor_scalar(out=acc2[:], in0=cosg[:, 0:800],
                                            scalar1=t_scal[:, base:base + 1],
                                            scalar2=t_scal[:, base + 1:base + 2],
                                            op0=AL.mult, op1=AL.add)
                    nc.vector.scalar_tensor_tensor(
                        out=acc2[:], in0=cosg[:, 1:801],
                        scalar=t_scal[:, base + 2:base + 3], in1=acc2[:],
                        op0=AL.mult, op1=AL.add)
                    nc.vector.scalar_tensor_tensor(
                        out=acc2[0:127, :], in0=cosgs1[0:127, 0:800],
                        scalar=t_scal[0:127, base + 3:base + 4],
                        in1=acc2[0:127, :], op0=AL.mult, op1=AL.add)
                    nc.vector.scalar_tensor_tensor(
                        out=acc2[0:127, :], in0=cosgs1[0:127, 1:801],
                        scalar=t_scal[0:127, base + 4:base + 5],
                        in1=acc2[0:127, :], op0=AL.mult, op1=AL.add)
                    if f == 0:
                        nc.vector.tensor_scalar(out=topf2[:], in0=acc2[:],
                                                scalar1=0.0, scalar2=None,
                                                op0=AL.max)
                    else:
                        nc.vector.tensor_tensor(out=topf2[:], in0=topf2[:],
                                                in1=acc2[:], op=AL.max)
                nc.vector.max(fe[:, 8:16], topf2[:])

                topf3 = vp.tile([128, 800], F16, tag="topf3")
                acc3 = vp.tile([128, 800], F16, tag="acc3")
                for f in range(32):
                    base = OFF3 + 10 * f
                    nc.vector.tensor_scalar(out=acc3[:], in0=cosg[:, 0:800],
                                            scalar1=t_scal[:, base:base + 1],
                                            scalar2=t_scal[:, base + 1:base + 2],
                                            op0=AL.mult, op1=AL.add)
                    for c in (1, 2):
                        nc.vector.scalar_tensor_tensor(
                            out=acc3[:], in0=cosg[:, c:800 + c],
                            scalar=t_scal[:, base + 1 + c:base + 2 + c],
                            in1=acc3[:], op0=AL.mult, op1=AL.add)
                    for a, csrc in ((1, cosgs1), (2, cosgs2)):
                        pmax = 128 - a
                        for c in (0, 1, 2):
                            col = base + 1 + 3 * a + c
                            nc.vector.scalar_tensor_tensor(
                                out=acc3[0:pmax, :], in0=csrc[0:pmax, c:800 + c],
                                scalar=t_scal[0:pmax, col:col + 1],
                                in1=acc3[0:pmax, :], op0=AL.mult, op1=AL.add)
                    if f == 0:
                        nc.vector.tensor_scalar(out=topf3[:], in0=acc3[:],
                                                scalar1=0.0, scalar2=None,
                                                op0=AL.max)
                    else:
                        nc.vector.tensor_tensor(out=topf3[:], in0=topf3[:],
                                                in1=acc3[:], op=AL.max)
                nc.vector.max(fe[:, 16:24], topf3[:])

                nc.sync.dma_start(out[g, :, :], fe[:])

    return out


# ---------------------------------------------------------------------------
# runtime
# ---------------------------------------------------------------------------

def _table_fp(emb):
    return (emb.shape, float(emb[0, 0]), float(emb[-1, -1]),
            float(emb[::971, ::7].sum()))


def _get_runtime(emb):
    fp = _table_fp(emb)
    if _rt.get("fp") == fp:
        return _rt
    import jax
    from jax.sharding import Mesh, PartitionSpec as P, NamedSharding
    from concourse.bass2jax import bass_jit, bass_shard_map

    devs = jax.devices()[:NCORES]
    mesh = Mesh(np.asarray(devs), ("core",))
    rep = NamedSharding(mesh, P())
    table = _make_table(emb)
    table_dev = jax.jit(lambda x: x, in_shardings=rep, out_shardings=rep)(table)
    table_dev.block_until_ready()

    if "fn" not in _rt:
        _rt["fn"] = bass_shard_map(
            bass_jit(_emit), mesh=mesh,
            in_specs=(P(), P("core"), P("core"), P("core"), P("core"),
                      P("core"), P("core")),
            out_specs=P("core"))
    _rt["table_dev"] = table_dev
    _rt["fp"] = fp
    return _rt


def kernel(qrls_words, doc_words, emb_table, idf_table,
           conv1_w, conv1_b, conv2_w, conv2_b, conv3_w, conv3_b,
           w1, b1, w2, b2, w3, b3):
    qw = np.asarray(qrls_words).astype(np.int64)
    dw = np.asarray(doc_words).astype(np.int64)
    emb = np.asarray(emb_table, np.float32)
    idf_t = np.asarray(idf_table, np.float32)
    B = qw.shape[0]
    assert B == B_TOTAL and qw.shape[1] == Q and dw.shape[1] == D

    rt = _get_runtime(emb)

    ilo_l, ihi_l, qlo_l, qhi_l, rq_l = [], [], [], [], []
    for k in range(NCORES):
        sl = slice(NB * k, NB * (k + 1))
        a, b_ = _doc_indices(dw[sl])
        c, d_ = _q_indices(qw[sl])
        ilo_l.append(a); ihi_l.append(b_); qlo_l.append(c); qhi_l.append(d_)
        qn = np.linalg.norm(emb[qw[sl]], axis=2)
        rq_l.append(_make_rq(qn))
    scal = _make_scal(np.asarray(conv1_w, np.float32), np.asarray(conv1_b, np.float32),
                      np.asarray(conv2_w, np.float32), np.asarray(conv2_b, np.float32),
                      np.asarray(conv3_w, np.float32), np.asarray(conv3_b, np.float32))

    feats = rt["fn"](rt["table_dev"],
                     np.concatenate(ilo_l, 0), np.concatenate(ihi_l, 0),
                     np.concatenate(qlo_l, 0), np.concatenate(qhi_l, 0),
                     np.tile(scal, (NCORES, 1)), np.concatenate(rq_l, 0))
    fnp = np.asarray(feats).astype(np.float32)     # [NCORES*NG, 128, 32]

    idf = idf_t[qw]                                # [B, 16]
    scores = np.zeros((B, Q, 13), np.float32)
    # (k, g, r) ordering of [NCORES*NG, 8, 16, 32] is exactly batch order
    f6 = fnp.reshape(B, Q, 32)
    scores[:, :, 0:2] = f6[:, :, 0:2]
    scores[:, :, 2:4] = f6[:, :, 8:10]
    scores[:, :, 4:6] = f6[:, :, 16:18]
    scores[:, :, 6:12] = f6[:, :, 24:30]
    scores[:, :, 12] = idf

    x = scores.reshape(B, -1)
    x = np.maximum(x @ np.asarray(w1, np.float32) + np.asarray(b1, np.float32), 0)
    x = np.maximum(x @ np.asarray(w2, np.float32) + np.asarray(b2, np.float32), 0)
    return x @ np.asarray(w3, np.float32) + np.asarray(b3, np.float32)


# revision 5
# speedup vs baseline: 2.0334x; 1.1734x over previous
"""COPACRR forward pass on 8 Trainium2 NeuronCores (Bass kernel).

Strategy
--------
Pure data parallel over the batch dim (16 batches per core). The frozen
embedding table is uploaded once (fp16, split + padded for int16 SWDGE
gather indices) and kept device-resident; per call only the word indices
and a few small scalar tables cross the tunnel (<1MB), plus a [NG,128,32]
feature tensor coming back per core. The per-core Bass kernel does:

  - gpsimd.dma_gather(transpose=True) of doc/query embeddings straight
    into [E-partition, doc-free] fp16 layout (two gathers per tensor: the
    50k-row table is split in half so indices fit int16; out-of-half
    indices point at zero rows and the two gathers are summed)
  - PE matmuls for the q x d dot products (fp32 PSUM)
  - doc/context norms via ACT Square + ones-vector PE reduction;
    sliding-window context sums via DVE tensor_tensor_scan + shifted sub
  - cosine normalisation with row-broadcast via a constant selection
    matmul; per-query scale folded in with scalar_tensor_tensor
  - the three PACRR convs as per-filter fused multiply-accumulate over
    shifted views (q-shifts via DMA-shifted copies), relu+filter-max by
    max-accumulation (the 1x1 conv collapses to its upper envelope,
    computed host-side)
  - all top-k via the DVE top-8 instruction (InstMax, keeps duplicates,
    matching jax.lax.top_k tie semantics)

The 13-feature MLP head (208->32->32->1) runs on the host in fp32.
"""
import numpy as np

Q = 16
D = 800
E = 300
EP = 384          # padded embedding cols (fp16 row = 768B, %256==0)
DP = 896          # padded doc count per batch (7*128)
NSPLIT = 25000    # table split for int16 gather indices
ZLO = NSPLIT      # zero row (local) in lo half
ZHI = 25001       # zero row (local) in hi half
NL1 = 12          # envelope line slots for the 1x1 conv
NSCAL = 2 * NL1 + 5 * 32 + 10 * 32
OFF2 = 2 * NL1
OFF3 = OFF2 + 5 * 32
import os as _os
NCORES = int(_os.environ.get("COPA_NCORES", "8"))
B_TOTAL = 128
NB = B_TOTAL // NCORES
NG = NB // 8

_rt = {}


# ---------------------------------------------------------------------------
# host-side prep
# ---------------------------------------------------------------------------

def _make_table(emb_table):
    T = np.zeros((50003, EP), np.float16)
    n = emb_table.shape[0]          # 50001
    T[0:NSPLIT, 0:E] = emb_table[0:NSPLIT]
    T[NSPLIT + 1:NSPLIT + 1 + (n - NSPLIT), 0:E] = emb_table[NSPLIT:n]
    return T


def _split_idx(w):
    lo = np.where(w < NSPLIT, w, ZLO).astype(np.int16)
    hi = np.where(w >= NSPLIT, w - NSPLIT, ZHI).astype(np.int16)
    return lo, hi


def _wrap(idx):
    b, n = idx.shape
    return np.ascontiguousarray(idx.reshape(b, n // 16, 16).transpose(2, 0, 1))


def _doc_indices(doc_words):
    nb = doc_words.shape[0]
    lo, hi = _split_idx(doc_words)
    lo_p = np.full((nb, DP), ZLO, np.int16)
    hi_p = np.full((nb, DP), ZHI, np.int16)
    lo_p[:, :D] = lo
    hi_p[:, :D] = hi
    return _wrap(lo_p), _wrap(hi_p)


def _q_indices(qrls_words):
    nb = qrls_words.shape[0]
    lo, hi = _split_idx(qrls_words.reshape(1, nb * Q))
    return _wrap(lo)[:, 0, :], _wrap(hi)[:, 0, :]


def _upper_envelope(ws, bs):
    best = {}
    for w, b in [(0.0, 0.0)] + list(zip(ws.tolist(), bs.tolist())):
        if w not in best or b > best[w]:
            best[w] = b
    pts = sorted(best.items())
    hull = []
    for p in pts:
        while len(hull) >= 2:
            o, a = hull[-2], hull[-1]
            if (a[0] - o[0]) * (p[1] - o[1]) - (a[1] - o[1]) * (p[0] - o[0]) >= 0:
                hull.pop()
            else:
                break
        hull.append(p)
    return hull


def _make_scal(c1w, c1b, c2w, c2b, c3w, c3b):
    s = np.zeros((16, NSCAL), np.float32)
    hull = _upper_envelope(c1w.reshape(32).astype(np.float64),
                           c1b.astype(np.float64))
    if len(hull) > NL1:          # extremely unlikely; keep best-slope spread
        raise RuntimeError(f"1x1 envelope needs {len(hull)} > {NL1} lines")
    for k, (w, b) in enumerate(hull):
        s[:, 2 * k] = w
        s[:, 2 * k + 1] = b
    qpos = np.arange(16)
    m1 = (qpos + 1 <= 15).astype(np.float32)
    m2 = (qpos + 2 <= 15).astype(np.float32)
    w2 = c2w.reshape(32, 2, 2)
    for f in range(32):
        base = OFF2 + 5 * f
        s[:, base + 0] = w2[f, 0, 0]
        s[:, base + 1] = c2b[f]
        s[:, base + 2] = w2[f, 0, 1]
        s[:, base + 3] = w2[f, 1, 0] * m1
        s[:, base + 4] = w2[f, 1, 1] * m1
    w3 = c3w.reshape(32, 3, 3)
    for f in range(32):
        base = OFF3 + 10 * f
        s[:, base + 0] = w3[f, 0, 0]
        s[:, base + 1] = c3b[f]
        s[:, base + 2] = w3[f, 0, 1]
        s[:, base + 3] = w3[f, 0, 2]
        s[:, base + 4] = w3[f, 1, 0] * m1
        s[:, base + 5] = w3[f, 1, 1] * m1
        s[:, base + 6] = w3[f, 1, 2] * m1
        s[:, base + 7] = w3[f, 2, 0] * m2
        s[:, base + 8] = w3[f, 2, 1] * m2
        s[:, base + 9] = w3[f, 2, 2] * m2
    return s


def _make_rq(qn):
    """qn [nb, 16] query norms -> [128, NG] per-partition reciprocal scale."""
    nb = qn.shape[0]
    rq = 1.0 / (qn + 1e-9)
    out = np.zeros((128, nb // 8), np.float32)
    for bl in range(nb):
        out[16 * (bl % 8):16 * (bl % 8) + 16, bl // 8] = rq[bl]
    return out


# ---------------------------------------------------------------------------
# bass kernel
# ---------------------------------------------------------------------------

def _emit(nc, table, ilo, ihi, qlo, qhi, scal16, rq):
    import concourse.mybir as mybir
    import concourse.tile as tile

    AL = mybir.AluOpType
    AF = mybir.ActivationFunctionType
    F16, F32, I16 = mybir.dt.float16, mybir.dt.float32, mybir.dt.int16
    HALVES = ((0, 512), (512, 800))
    NQ = 16 * NB
    NI = DP // 16
    out = nc.dram_tensor("feats", [NG, 128, 32], F16, kind="ExternalOutput")

    with tile.TileContext(nc) as tc:
        with (
            tc.tile_pool(name="const", bufs=1) as cp,
            tc.tile_pool(name="batch", bufs=2) as bp,
            tc.tile_pool(name="grp", bufs=2) as gp,
            tc.tile_pool(name="conv", bufs=2) as vp,
            tc.tile_pool(name="praw", bufs=1, space="PSUM") as ppraw,
            tc.tile_pool(name="pdn", bufs=1, space="PSUM") as ppdn,
            tc.tile_pool(name="pcn", bufs=1, space="PSUM") as ppcn,
            tc.tile_pool(name="pbc", bufs=1, space="PSUM") as ppbc,
        ):
            t_ilo = cp.tile([128, NB, NI], I16)
            t_ihi = cp.tile([128, NB, NI], I16)
            t_qlo = cp.tile([128, NB], I16)
            t_qhi = cp.tile([128, NB], I16)
            # HW's SWDGE idx stream reads partitions 16..31; CoreSim reads 0..15.
            for t, src in ((t_ilo, ilo), (t_ihi, ihi), (t_qlo, qlo), (t_qhi, qhi)):
                nc.vector.memset(t[:], 0)
                nc.sync.dma_start(t[0:16], src[:])
                nc.sync.dma_start(t[16:32], src[:])
            t_scal16 = cp.tile([16, NSCAL], F32)
            nc.sync.dma_start(t_scal16[:], scal16[:])
            t_rq = cp.tile([128, NG], F32)
            nc.sync.dma_start(t_rq[:], rq[:])

            t_scal = cp.tile([128, NSCAL], F32)
            for k in range(8):
                nc.sync.dma_start(t_scal[16 * k:16 * k + 16, :], scal16[:])

            # selR8[k, m] = (m // 16 == k): broadcasts row k to partitions 16k..
            t_sel8 = cp.tile([8, 128], F32)
            nc.gpsimd.memset(t_sel8[:], 1.0)
            nc.gpsimd.affine_select(out=t_sel8[:], in_=t_sel8[:],
                                    compare_op=AL.is_ge, fill=0.0, base=0,
                                    pattern=[[1, 128]], channel_multiplier=-16)
            nc.gpsimd.affine_select(out=t_sel8[:], in_=t_sel8[:],
                                    compare_op=AL.is_ge, fill=0.0, base=15,
                                    pattern=[[-1, 128]], channel_multiplier=16)

            t_ones128 = cp.tile([128, 1], F16)
            nc.vector.memset(t_ones128[:], 1.0)

            t_qloT = cp.tile([128, 3, NQ], F16, tag="qT")
            t_qhiT = cp.tile([128, 3, NQ], F16, tag="qTh")
            nc.gpsimd.dma_gather(t_qloT[:], table[0:NSPLIT + 1, :], t_qlo[:],
                                 NQ, NQ, EP, transpose=True)
            nc.gpsimd.dma_gather(t_qhiT[:], table[NSPLIT + 1:50003, :], t_qhi[:],
                                 NQ, NQ, EP, transpose=True)
            nc.vector.tensor_tensor(out=t_qloT[:], in0=t_qloT[:], in1=t_qhiT[:],
                                    op=AL.add)
            t_qT = t_qloT

            for g in range(NG):
                raws = gp.tile([128, 800], F32, tag="raws")
                sdn = gp.tile([8, 800], F32, tag="sdn")
                scn = gp.tile([8, 800], F32, tag="scn")

                for r in range(8):
                    bl = 8 * g + r
                    dlo = bp.tile([128, 3, DP], F16, tag="dlo")
                    dhi = bp.tile([128, 3, DP], F16, tag="dhi")
                    nc.gpsimd.dma_gather(dlo[:], table[0:NSPLIT + 1, :],
                                         t_ilo[:, bl, :], DP, DP, EP,
                                         transpose=True)
                    nc.gpsimd.dma_gather(dhi[:], table[NSPLIT + 1:50003, :],
                                         t_ihi[:, bl, :], DP, DP, EP,
                                         transpose=True)
                    nc.vector.tensor_tensor(out=dlo[:], in0=dlo[:], in1=dhi[:],
                                            op=AL.add)

                    ps_raw = ppraw.tile([16, 800], F32, tag="praw")
                    for c in range(3):
                        for (s0, s1) in HALVES:
                            nc.tensor.matmul(
                                ps_raw[:, s0:s1],
                                t_qT[:, c, 16 * bl:16 * bl + 16],
                                dlo[:, c, s0:s1],
                                start=(c == 0), stop=(c == 2))
                    raw_st = bp.tile([16, 800], F32, tag="raw_st")
                    nc.vector.tensor_copy(raw_st[:], ps_raw[:])
                    nc.sync.dma_start(raws[16 * r:16 * r + 16, :], raw_st[:])

                    sq = bp.tile([128, 3, D], F16, tag="sq")
                    nc.scalar.activation(sq[:], dlo[:, :, 0:D], AF.Square)
                    ps_dn = ppdn.tile([1, 800], F32, tag="pdn")
                    for c in range(3):
                        for (s0, s1) in HALVES:
                            nc.tensor.matmul(
                                ps_dn[:, s0:s1], t_ones128[:],
                                sq[:, c, s0:s1], start=(c == 0), stop=(c == 2))
                    dn_st = bp.tile([1, 800], F32, tag="dn_st")
                    nc.vector.tensor_copy(dn_st[:], ps_dn[:])
                    nc.sync.dma_start(sdn[r:r + 1, :], dn_st[:])

                    csum = bp.tile([128, 3, 809], F32, tag="csum")
                    nc.vector.memset(csum[:, :, 0:5], 0.0)
                    for c in range(3):
                        nc.vector.tensor_tensor_scan(
                            out=csum[:, c, 5:805], data0=dlo[:, c, 0:D],
                            data1=dlo[:, c, 0:D], initial=0.0,
                            op0=AL.add, op1=AL.bypass)
                    ctxs = bp.tile([128, 3, D], F32, tag="ctxs")
                    nc.vector.tensor_tensor(out=ctxs[:, :, 0:797],
                                            in0=csum[:, :, 8:805],
                                            in1=csum[:, :, 0:797],
                                            op=AL.subtract)
                    for c in range(3):
                        nc.vector.tensor_scalar(
                            out=ctxs[:, c, 797:800], in0=csum[:, c, 797:800],
                            scalar1=-1.0, scalar2=csum[:, c, 804:805],
                            op0=AL.mult, op1=AL.add)
                    sqc = bp.tile([128, 3, D], F16, tag="sqc")
                    nc.scalar.activation(sqc[:], ctxs[:], AF.Square)
                    ps_cn = ppcn.tile([1, 800], F32, tag="pcn")
                    for c in range(3):
                        for (s0, s1) in HALVES:
                            nc.tensor.matmul(
                                ps_cn[:, s0:s1], t_ones128[:],
                                sqc[:, c, s0:s1], start=(c == 0), stop=(c == 2))
                    cn_st = bp.tile([1, 800], F32, tag="cn_st")
                    nc.vector.tensor_copy(cn_st[:], ps_cn[:])
                    nc.sync.dma_start(scn[r:r + 1, :], cn_st[:])

                dnr = gp.tile([8, 800], F32, tag="dnr")
                rrd = gp.tile([8, 800], F32, tag="rrd")
                rrc = gp.tile([8, 800], F32, tag="rrc")
                nc.scalar.activation(dnr[:], sdn[:], AF.Sqrt)
                nc.vector.tensor_scalar(out=dnr[:], in0=dnr[:], scalar1=1e-9,
                                        scalar2=None, op0=AL.add)
                nc.vector.reciprocal(rrd[:], dnr[:])
                nc.scalar.activation(dnr[:], scn[:], AF.Sqrt)
                nc.vector.tensor_scalar(out=dnr[:], in0=dnr[:], scalar1=9e-9,
                                        scalar2=None, op0=AL.add)
                nc.vector.reciprocal(rrc[:], dnr[:])

                ps_bc = ppbc.tile([128, 800], F32, tag="pbc")
                for (s0, s1) in HALVES:
                    nc.tensor.matmul(ps_bc[:, s0:s1], t_sel8[:], rrd[:, s0:s1],
                                     start=True, stop=True)
                cosg = gp.tile([128, 804], F16, tag="cosg")
                nc.vector.memset(cosg[:, 800:804], 0.0)
                nc.vector.scalar_tensor_tensor(
                    out=cosg[:, 0:800], in0=raws[:], scalar=t_rq[:, g:g + 1],
                    in1=ps_bc[:], op0=AL.mult, op1=AL.mult)

                csn = gp.tile([128, 809], F32, tag="csn")
                nc.vector.memset(csn[:, 0:5], 0.0)
                nc.vector.tensor_tensor_scan(
                    out=csn[:, 5:805], data0=raws[:], data1=raws[:],
                    initial=0.0, op0=AL.add, op1=AL.bypass)
                ctxn = gp.tile([128, 800], F32, tag="ctxn")
                nc.vector.tensor_tensor(out=ctxn[:, 0:797], in0=csn[:, 8:805],
                                        in1=csn[:, 0:797], op=AL.subtract)
                nc.vector.tensor_scalar(out=ctxn[:, 797:800],
                                        in0=csn[:, 797:800],
                                        scalar1=-1.0, scalar2=csn[:, 804:805],
                                        op0=AL.mult, op1=AL.add)
                ps_bc2 = ppbc.tile([128, 800], F32, tag="pbc")
                for (s0, s1) in HALVES:
                    nc.tensor.matmul(ps_bc2[:, s0:s1], t_sel8[:], rrc[:, s0:s1],
                                     start=True, stop=True)
                coc = gp.tile([128, 800], F16, tag="coc")
                nc.vector.scalar_tensor_tensor(
                    out=coc[:], in0=ctxn[:], scalar=t_rq[:, g:g + 1],
                    in1=ps_bc2[:], op0=AL.mult, op1=AL.mult)

                cosgs1 = gp.tile([128, 804], F16, tag="cosgs1")
                cosgs2 = gp.tile([128, 804], F16, tag="cosgs2")
                nc.sync.dma_start(cosgs1[0:127, :], cosg[1:128, :])
                nc.sync.dma_start(cosgs2[0:126, :], cosg[2:128, :])

                fe = gp.tile([128, 32], F16, tag="fe")
                nc.vector.max(fe[:, 24:32], coc[:])

                topf1 = vp.tile([128, 800], F16, tag="topf1")
                tmp1 = vp.tile([128, 800], F16, tag="tmp1")
                nc.vector.tensor_scalar(out=topf1[:], in0=cosg[:, 0:800],
                                        scalar1=t_scal[:, 0:1],
                                        scalar2=t_scal[:, 1:2],
                                        op0=AL.mult, op1=AL.add)
                for k in range(1, NL1):
                    nc.vector.tensor_scalar(out=tmp1[:], in0=cosg[:, 0:800],
                                            scalar1=t_scal[:, 2 * k:2 * k + 1],
                                            scalar2=t_scal[:, 2 * k + 1:2 * k + 2],
                                            op0=AL.mult, op1=AL.add)
                    nc.vector.tensor_tensor(out=topf1[:], in0=topf1[:],
                                            in1=tmp1[:], op=AL.max)
                nc.vector.max(fe[:, 0:8], topf1[:])

                topf2 = vp.tile([128, 800], F16, tag="topf2")
                acc2 = vp.tile([128, 800], F16, tag="acc2")
                for f in range(32):
                    base = OFF2 + 5 * f
                    nc.vector.tensor_scalar(out=acc2[:], in0=cosg[:, 0:800],
                                            scalar1=t_scal[:, base:base + 1],
                                            scalar2=t_scal[:, base + 1:base + 2],
                                            op0=AL.mult, op1=AL.add)
                    nc.vector.scalar_tensor_tensor(
                        out=acc2[:], in0=cosg[:, 1:801],
                        scalar=t_scal[:, base + 2:base + 3], in1=acc2[:],
                        op0=AL.mult, op1=AL.add)
                    nc.vector.scalar_tensor_tensor(
                        out=acc2[0:127, :], in0=cosgs1[0:127, 0:800],
                        scalar=t_scal[0:127, base + 3:base + 4],
                        in1=acc2[0:127, :], op0=AL.mult, op1=AL.add)
                    nc.vector.scalar_tensor_tensor(
                        out=acc2[0:127, :], in0=cosgs1[0:127, 1:801],
                        scalar=t_scal[0:127, base + 4:base + 5],
                        in1=acc2[0:127, :], op0=AL.mult, op1=AL.add)
                    if f == 0:
                        nc.vector.tensor_scalar(out=topf2[:], in0=acc2[:],
                                                scalar1=0.0, scalar2=None,
                                                op0=AL.max)
                    else:
                        nc.vector.tensor_tensor(out=topf2[:], in0=topf2[:],
                                                in1=acc2[:], op=AL.max)
                nc.vector.max(fe[:, 8:16], topf2[:])

                topf3 = vp.tile([128, 800], F16, tag="topf3")
                acc3 = vp.tile([128, 800], F16, tag="acc3")
                for f in range(32):
                    base = OFF3 + 10 * f
                    nc.vector.tensor_scalar(out=acc3[:], in0=cosg[:, 0:800],
                                            scalar1=t_scal[:, base:base + 1],
                                            scalar2=t_scal[:, base + 1:base + 2],
                                            op0=AL.mult, op1=AL.add)
                    for c in (1, 2):
                        nc.vector.scalar_tensor_tensor(
                            out=acc3[:], in0=cosg[:, c:800 + c],
                            scalar=t_scal[:, base + 1 + c:base + 2 + c],
                            in1=acc3[:], op0=AL.mult, op1=AL.add)
                    for a, csrc in ((1, cosgs1), (2, cosgs2)):
                        pmax = 128 - a
                        for c in (0, 1, 2):
                            col = base + 1 + 3 * a + c
                            nc.vector.scalar_tensor_tensor(
                                out=acc3[0:pmax, :], in0=csrc[0:pmax, c:800 + c],
                                scalar=t_scal[0:pmax, col:col + 1],
                                in1=acc3[0:pmax, :], op0=AL.mult, op1=AL.add)
                    if f == 0:
                        nc.vector.tensor_scalar(out=topf3[:], in0=acc3[:],
                                                scalar1=0.0, scalar2=None,
                                                op0=AL.max)
                    else:
                        nc.vector.tensor_tensor(out=topf3[:], in0=topf3[:],
                                                in1=acc3[:], op=AL.max)
                nc.vector.max(fe[:, 16:24], topf3[:])

                nc.sync.dma_start(out[g, :, :], fe[:])

    return out


# ---------------------------------------------------------------------------
# runtime
# ---------------------------------------------------------------------------

def _table_fp(emb):
    return (emb.shape, float(emb[0, 0]), float(emb[-1, -1]),
            float(emb[::971, ::7].sum()))


def _get_runtime(emb):
    fp = _table_fp(emb)
    if _rt.get("fp") == fp:
        return _rt
    import jax
    from jax.sharding import Mesh, PartitionSpec as P, NamedSharding
    from concourse.bass2jax import bass_jit, bass_shard_map

    devs = jax.devices()[:NCORES]
    mesh = Mesh(np.asarray(devs), ("core",))
    rep = NamedSharding(mesh, P())
    table = _make_table(emb)
    table_dev = jax.jit(lambda x: x, in_shardings=rep, out_shardings=rep)(table)
    table_dev.block_until_ready()

    if "fn" not in _rt:
        _rt["fn"] = bass_shard_map(
            bass_jit(_emit), mesh=mesh,
            in_specs=(P(), P("core"), P("core"), P("core"), P("core"),
                      P("core"), P("core")),
            out_specs=P("core"))
    _rt["table_dev"] = table_dev
    _rt["fp"] = fp
    return _rt


def kernel(qrls_words, doc_words, emb_table, idf_table,
           conv1_w, conv1_b, conv2_w, conv2_b, conv3_w, conv3_b,
           w1, b1, w2, b2, w3, b3):
    qw = np.asarray(qrls_words).astype(np.int64)
    dw = np.asarray(doc_words).astype(np.int64)
    emb = np.asarray(emb_table, np.float32)
    idf_t = np.asarray(idf_table, np.float32)
    B = qw.shape[0]
    assert B == B_TOTAL and qw.shape[1] == Q and dw.shape[1] == D

    rt = _get_runtime(emb)

    # doc indices, all batches at once -> per-core stacked [NCORES*16, NB, DP/16]
    lo, hi = _split_idx(dw)
    lo_p = np.full((B, DP), ZLO, np.int16)
    hi_p = np.full((B, DP), ZHI, np.int16)
    lo_p[:, :D] = lo
    hi_p[:, :D] = hi

    def _stack_doc(x):
        w = x.reshape(B, DP // 16, 16).transpose(2, 0, 1)          # [16, B, NI]
        return np.ascontiguousarray(
            w.reshape(16, NCORES, NB, DP // 16).transpose(1, 0, 2, 3)
            .reshape(16 * NCORES, NB, DP // 16))

    qlo, qhi = _split_idx(qw.reshape(NCORES, NB * Q))

    def _stack_q(x):                                               # [NCORES, NB*16]
        return np.ascontiguousarray(
            x.reshape(NCORES, NB * Q // 16, 16).transpose(0, 2, 1)
            .reshape(16 * NCORES, NB))

    qn = np.linalg.norm(emb[qw], axis=2)                           # [B, 16]
    rqv = (1.0 / (qn + 1e-9)).astype(np.float32)
    rq_all = np.ascontiguousarray(
        rqv.reshape(NCORES, NG, 8, Q).transpose(0, 2, 3, 1)
        .reshape(128 * NCORES, NG))

    scal = _make_scal(np.asarray(conv1_w, np.float32), np.asarray(conv1_b, np.float32),
                      np.asarray(conv2_w, np.float32), np.asarray(conv2_b, np.float32),
                      np.asarray(conv3_w, np.float32), np.asarray(conv3_b, np.float32))

    feats = rt["fn"](rt["table_dev"],
                     _stack_doc(lo_p), _stack_doc(hi_p),
                     _stack_q(qlo), _stack_q(qhi),
                     np.tile(scal, (NCORES, 1)), rq_all)
    fnp = np.asarray(feats).astype(np.float32)     # [NCORES*NG, 128, 32]

    idf = idf_t[qw]                                # [B, 16]
    scores = np.zeros((B, Q, 13), np.float32)
    # (k, g, r) ordering of [NCORES*NG, 8, 16, 32] is exactly batch order
    f6 = fnp.reshape(B, Q, 32)
    scores[:, :, 0:2] = f6[:, :, 0:2]
    scores[:, :, 2:4] = f6[:, :, 8:10]
    scores[:, :, 4:6] = f6[:, :, 16:18]
    scores[:, :, 6:12] = f6[:, :, 24:30]
    scores[:, :, 12] = idf

    x = scores.reshape(B, -1)
    x = np.maximum(x @ np.asarray(w1, np.float32) + np.asarray(b1, np.float32), 0)
    x = np.maximum(x @ np.asarray(w2, np.float32) + np.asarray(b2, np.float32), 0)
    return x @ np.asarray(w3, np.float32) + np.asarray(b3, np.float32)
